# revision 13
# baseline (speedup 1.0000x reference)
"""Per-sample 256-bin histogram -> broadcast [B,256,256], Trainium2 Bass kernel.

Input : x int32 [64, 786432], values in [0, 256)
Output: f32 [64, 256, 256] where out[b, i, j] = count(x[b, :] == i)

Sharding: pure data parallel, 8 rows per core across 8 NeuronCores.

Per-core algorithm (cumulative-threshold decomposition, grouped outer
products, v2):
  J2[a, b]   = sum_n [x_n < 16(a+1)] * [x_n & 15 <= b]   (cumulative in BOTH)
  hist[16a+b] = dda ddb J2  (2-D finite difference)
  Every mask plane is ONE instruction on ONE of three engines:
  - DVE   : tensor_scalar is_lt (h-planes) / chained and+is_le (l-planes),
            int16 in -> bf16 out, 4x mode (~327 ns/plane @ T=1024).
  - ACT   : Sign activation (+-1 encoding, fixed up in the epilogue),
            one op per plane (~1147 ns/plane). ACT also casts the
            contiguously-DMA'd int32 input to int16 (the int16-strided
            DMA variant shatters into 2-byte packets - catastrophic).
  - GPSIMD: same tensor_scalar chains, n_gps planes (port-shared w/ DVE).
  PE accumulates [128,128] PSUM outer products with 8 element groups per
  matmul (1024 elements / matmul instruction).
  Epilogue per row: diag blocks -> J' [16,16]; ACT (+-1) rows fixed via
  J2 = (J' + Sigma_l)/2 where Sigma_l = J'[15,:]; 2-D difference;
  partition-reshape to [128,2]; broadcast multiply; write out.
  Counts are integer-exact in f32 (cumulative counts < 2^24).
"""

import os
import sys

import numpy as np

sys.path.insert(0, "/opt/trn_rl_repo")

B = 64
N = 786432
NCORES = 8
ROWS_PER_CORE = B // NCORES
LEVELS = 256
P = 128

T = int(os.environ.get("K_T", "1024"))  # columns per tile
G = 8  # element groups per matmul
C = T // G  # matmul chunks per tile
TILES = N // (P * T)
assert TILES * P * T == N and C * G == T

NACC = int(os.environ.get("K_NACC", "1"))
N_ACT = int(os.environ.get("K_ACT", "6"))  # h-planes on ACT (Sign, +-1)
N_GPS = int(os.environ.get("K_GPS", "0"))  # h-planes on GPSIMD
MDT = os.environ.get("K_MDT", "bf16")  # mask dtype: bf16 | fp8
assert N_ACT + N_GPS <= 16

_cache = {}


def _build_program(rows=None):
    import concourse.bacc as bacc
    from concourse import mybir
    from concourse import tile

    alu = mybir.AluOpType
    dt = mybir.dt
    act = mybir.ActivationFunctionType

    rows = ROWS_PER_CORE if rows is None else rows

    nc = bacc.Bacc(
        "TRN2",
        target_bir_lowering=False,
        debug=False,
        num_devices=NCORES,
    )
    x_dram = nc.dram_tensor("x", [rows, N], dt.int32, kind="ExternalInput")
    out_dram = nc.dram_tensor(
        "out", [rows, LEVELS, LEVELS], dt.float32, kind="ExternalOutput"
    )

    xv = x_dram.ap().rearrange("r (t p f) -> r t p f", p=P, f=T)
    ov = out_dram.ap()

    # h-plane engine assignment: a = 0..15
    #   DVE  : a in [0, 16-N_ACT-N_GPS)
    #   GPS  : a in [16-N_ACT-N_GPS, 16-N_ACT)
    #   ACT  : a in [16-N_ACT, 16)   (+-1 encoding; includes a=15 == const +1)
    a0_gps = 16 - N_ACT - N_GPS
    a0_act = 16 - N_ACT

    with tile.TileContext(nc) as tc:
        with (
            tc.tile_pool(name="xin", bufs=2) as xpool,
            tc.tile_pool(name="x16", bufs=2) as x16pool,
            tc.tile_pool(name="mask", bufs=2) as mpool,
            tc.tile_pool(name="acc", bufs=2, space="PSUM") as ppool,
            tc.tile_pool(name="bc", bufs=2, space="PSUM") as bcpool,
            tc.tile_pool(name="epi", bufs=2) as epool,
            tc.tile_pool(name="const", bufs=1) as cpool,
        ):
            ones_t = cpool.tile([P, LEVELS], dt.float32)
            nc.vector.memset(ones_t[:], 1.0)
            # stationary weight row for the epilogue broadcast matmul:
            # w_a = 1 for +-1-encoded (ACT) rows, 0 for {0,2}-encoded rows
            wvec16 = cpool.tile([1, 16], dt.float32)
            nc.vector.memset(wvec16[:], 0.0)
            if N_ACT:
                nc.vector.memset(wvec16[:, a0_act:16], 1.0)
            # per-partition bias constants for the ACT Sign h-planes
            bias_act = cpool.tile([P, N_ACT if N_ACT else 1], dt.float32)
            for i in range(N_ACT):
                a = a0_act + i
                nc.vector.memset(bias_act[:, i : i + 1], float(16 * (a + 1)) - 0.5)

            for r in range(rows):
                psums = [
                    ppool.tile([P, P], dt.float32, tag=f"ps{k}", name=f"ps{k}")
                    for k in range(NACC)
                ]
                for t in range(TILES):
                    qs = T // 4
                    x32 = xpool.tile([P, T], dt.int32, tag="x32")
                    for q in range(4):
                        nc.sync.dma_start(
                            out=x32[:, q * qs : (q + 1) * qs],
                            in_=xv[r, t, :, q * qs : (q + 1) * qs],
                        )
                    x16 = x16pool.tile([P, T], dt.int16, tag="x16")
                    nc.scalar.copy(out=x16[:], in_=x32[:])

                    # element (p, i) -> group g = i % G, chunk c = i // G
                    xg = x16[:].rearrange("p (c g) -> p c g", g=G)

                    mdt = dt.bfloat16 if MDT == "bf16" else dt.float8e4
                    hm = mpool.tile([P, C, 16, G], mdt, tag="hm")
                    lm = mpool.tile([P, C, 16, G], mdt, tag="lm")

                    # --- l-planes: [x & 15 <= b], 0/1 (hw forbids mixing
                    # bitwise op0 with arith op1, so extract xl first)
                    xl = x16pool.tile([P, T], dt.int16, tag="xl")
                    nc.vector.tensor_scalar(
                        out=xl[:], in0=x16[:], scalar1=15, scalar2=None,
                        op0=alu.bitwise_and,
                    )
                    xlg = xl[:].rearrange("p (c g) -> p c g", g=G)
                    for b in range(16):
                        nc.vector.tensor_scalar(
                            out=lm[:, :, b, :], in0=xlg,
                            scalar1=b, scalar2=None,
                            op0=alu.is_le,
                        )

                    # --- h-planes: [x < 16(a+1)], {0,2}-encoded on DVE/GPS
                    # (uniform epilogue fix: J2 = (J' + w*Sigma_l)/2)
                    for a in range(16):
                        if a < a0_gps:
                            nc.vector.tensor_scalar(
                                out=hm[:, :, a, :], in0=xg,
                                scalar1=16 * (a + 1), scalar2=2,
                                op0=alu.is_lt, op1=alu.mult,
                            )
                        elif a < a0_act:
                            nc.gpsimd.tensor_scalar(
                                out=hm[:, :, a, :], in0=xg,
                                scalar1=16 * (a + 1), scalar2=2,
                                op0=alu.is_lt, op1=alu.mult,
                            )
                        else:
                            # +-1 == Sign(16(a+1) - 0.5 - x); fixed in epilogue
                            nc.scalar.activation(
                                hm[:, :, a, :], xg, act.Sign,
                                bias=bias_act[:, a - a0_act : a - a0_act + 1],
                                scale=-1.0,
                            )

                    for c in range(C):
                        k = c % NACC
                        nc.tensor.matmul(
                            out=psums[k][:],
                            lhsT=hm[:, c, :, :],
                            rhs=lm[:, c, :, :],
                            start=(t == 0 and c < NACC),
                            stop=(t == TILES - 1 and c >= C - NACC),
                        )

                # --- epilogue for row r ---
                # only one PSUM operand allowed per DVE instruction; use the
                # ACT engine for the PSUM drain to keep DVE free
                hsum = epool.tile([P, P], dt.float32, tag="hsum")
                nc.scalar.copy(out=hsum[:], in_=psums[0][:])
                for k in range(1, NACC):
                    nc.vector.tensor_tensor(
                        out=hsum[:], in0=hsum[:], in1=psums[k][:],
                        op=alu.add,
                    )

                # valid block for group g lives at psum[a*G+g, b*G+g]
                hv = hsum[:].rearrange("(a gi) (l gj) -> a gi l gj", gi=G, gj=G)
                tmp = epool.tile([16, 16, G], dt.float32, tag="tmp")
                for g in range(G):
                    nc.sync.dma_start(out=tmp[:, :, g], in_=hv[:, g, :, g])
                jmat = epool.tile([16, 16], dt.float32, tag="jmat")
                nc.vector.tensor_reduce(
                    out=jmat[:], in_=tmp[:], axis=mybir.AxisListType.X,
                    op=alu.add,
                )

                # uniform fix: J2 = (J' + w_a * Sigma_l)/2 with w_a baked
                # into the broadcast matmul stationary. Sigma_l = J'[15,:]
                # (row 15 is the const +1 ACT plane).
                sig = epool.tile([1, 16], dt.float32, tag="sig")
                nc.sync.dma_start(out=sig[:], in_=jmat[15:16, :])
                bc = bcpool.tile([16, 16], dt.float32, tag="bc")
                nc.tensor.matmul(
                    out=bc[:],
                    lhsT=wvec16[:],
                    rhs=sig[:],
                    start=True, stop=True,
                )
                nc.vector.tensor_tensor(
                    out=jmat[:], in0=jmat[:], in1=bc[:], op=alu.add,
                )
                nc.vector.tensor_scalar(
                    out=jmat[:], in0=jmat[:],
                    scalar1=0.5, scalar2=None, op0=alu.mult,
                )

                # difference along a (partition dim): K[a] = J2[a] - J2[a-1]
                jshift = epool.tile([16, 16], dt.float32, tag="jshift")
                nc.vector.memset(jshift[0:1, :], 0.0)
                nc.sync.dma_start(out=jshift[1:16, :], in_=jmat[0:15, :])
                kmat = epool.tile([16, 16], dt.float32, tag="kmat")
                nc.vector.tensor_tensor(
                    out=kmat[:], in0=jmat[:], in1=jshift[:],
                    op=alu.subtract,
                )
                # difference along b (free dim): hist16[:, b] = K[b] - K[b-1]
                hist16 = epool.tile([16, 16], dt.float32, tag="h16")
                nc.vector.tensor_copy(out=hist16[:, 0:1], in_=kmat[:, 0:1])
                nc.vector.tensor_tensor(
                    out=hist16[:, 1:16], in0=kmat[:, 1:16], in1=kmat[:, 0:15],
                    op=alu.subtract,
                )

                histcol = epool.tile([P, 2], dt.float32, tag="hcol")
                nc.sync.dma_start(out=histcol[:, 0:1], in_=hist16[0:8, :])
                nc.sync.dma_start(out=histcol[:, 1:2], in_=hist16[8:16, :])

                for half in range(2):
                    bt = epool.tile([P, LEVELS], dt.float32, tag="bt")
                    nc.scalar.mul(bt[:], ones_t[:], histcol[:, half : half + 1])
                    nc.sync.dma_start(
                        out=ov[r, half * P : (half + 1) * P, :], in_=bt[:]
                    )

    nc.compile()
    return nc


def _get_program(rows=None):
    key = ("nc", rows)
    if key not in _cache:
        _cache[key] = _build_program(rows)
    return _cache[key]


def kernel(x: np.ndarray) -> np.ndarray:
    from concourse.bass_utils import run_bass_kernel_spmd

    x = np.ascontiguousarray(np.asarray(x), dtype=np.int32)
    assert x.shape == (B, N), x.shape

    nc = _get_program()
    in_maps = [
        {"x": x[c * ROWS_PER_CORE : (c + 1) * ROWS_PER_CORE]} for c in range(NCORES)
    ]
    res = run_bass_kernel_spmd(nc, in_maps, core_ids=list(range(NCORES)))
    out = np.concatenate([res.results[c]["out"] for c in range(NCORES)], axis=0)
    return out.astype(np.float32)


# revision 22
# speedup vs baseline: 1.4758x; 1.4758x over previous
"""Per-sample 256-bin histogram -> broadcast [B,256,256], Trainium2 Bass kernel.

Input : x int32 [64, 786432], values in [0, 256)
Output: f32 [64, 256, 256] where out[b, i, j] = count(x[b, :] == i)

Sharding: pure data parallel, 8 rows per core across 8 NeuronCores.

Per-core algorithm (cumulative-threshold decomposition, grouped outer
products, v2):
  J2[a, b]   = sum_n [x_n < 16(a+1)] * [x_n & 15 <= b]   (cumulative in BOTH)
  hist[16a+b] = dda ddb J2  (2-D finite difference)
  Every mask plane is ONE instruction on ONE of three engines:
  - DVE   : tensor_scalar is_lt (h-planes) / chained and+is_le (l-planes),
            int16 in -> bf16 out, 4x mode (~327 ns/plane @ T=1024).
  - ACT   : Sign activation (+-1 encoding, fixed up in the epilogue),
            one op per plane (~1147 ns/plane). ACT also casts the
            contiguously-DMA'd int32 input to int16 (the int16-strided
            DMA variant shatters into 2-byte packets - catastrophic).
  - GPSIMD: same tensor_scalar chains, n_gps planes (port-shared w/ DVE).
  PE accumulates [128,128] PSUM outer products with 8 element groups per
  matmul (1024 elements / matmul instruction).
  Epilogue per row: diag blocks -> J' [16,16]; ACT (+-1) rows fixed via
  J2 = (J' + Sigma_l)/2 where Sigma_l = J'[15,:]; 2-D difference;
  partition-reshape to [128,2]; broadcast multiply; write out.
  Counts are integer-exact in f32 (cumulative counts < 2^24).
"""

import os
import sys

import numpy as np

sys.path.insert(0, "/opt/trn_rl_repo")

B = 64
N = 786432
NCORES = 8
ROWS_PER_CORE = B // NCORES
LEVELS = 256
P = 128

T = int(os.environ.get("K_T", "1024"))  # columns per tile
G = 8  # element groups per matmul
C = T // G  # matmul chunks per tile
TILES = N // (P * T)
assert TILES * P * T == N and C * G == T

NACC = int(os.environ.get("K_NACC", "1"))
N_ACT = int(os.environ.get("K_ACT", "6"))  # h-planes on ACT (Sign, +-1)
N_GPS = int(os.environ.get("K_GPS", "0"))  # h-planes on GPSIMD
MDT = os.environ.get("K_MDT", "bf16")  # mask dtype: bf16 | fp8
assert N_ACT + N_GPS <= 16

_cache = {}


def _build_program(rows=None):
    import concourse.bacc as bacc
    from concourse import mybir
    from concourse import tile

    alu = mybir.AluOpType
    dt = mybir.dt
    act = mybir.ActivationFunctionType

    rows = ROWS_PER_CORE if rows is None else rows

    nc = bacc.Bacc(
        "TRN2",
        target_bir_lowering=False,
        debug=False,
        num_devices=NCORES,
    )
    x_dram = nc.dram_tensor("x", [rows, N], dt.int32, kind="ExternalInput")
    out_dram = nc.dram_tensor(
        "out", [rows, LEVELS, LEVELS], dt.float32, kind="ExternalOutput"
    )

    xv = x_dram.ap().rearrange("r (t p f) -> r t p f", p=P, f=T)
    ov = out_dram.ap()

    # h-plane engine assignment: a = 0..15
    #   DVE  : a in [0, 16-N_ACT-N_GPS)
    #   GPS  : a in [16-N_ACT-N_GPS, 16-N_ACT)
    #   ACT  : a in [16-N_ACT, 16)   (+-1 encoding; includes a=15 == const +1)
    a0_gps = 16 - N_ACT - N_GPS
    a0_act = 16 - N_ACT

    with tile.TileContext(nc) as tc:
        with (
            tc.tile_pool(name="xin", bufs=2 if T <= 1024 else 1) as xpool,
            tc.tile_pool(name="x16", bufs=2) as x16pool,
            tc.tile_pool(name="xl", bufs=2 if T <= 1024 else 1) as xlpool,
            tc.tile_pool(name="mask", bufs=2) as mpool,
            tc.tile_pool(name="acc", bufs=2, space="PSUM") as ppool,
            tc.tile_pool(name="bc", bufs=2, space="PSUM") as bcpool,
            tc.tile_pool(name="epi", bufs=2 if T <= 1024 else 1) as epool,
            tc.tile_pool(name="const", bufs=1) as cpool,
        ):
            ones_t = cpool.tile([P, LEVELS], dt.bfloat16)
            nc.vector.memset(ones_t[:], 1.0)
            # stationary weight row for the epilogue broadcast matmul:
            # w_a = 1 for +-1-encoded (ACT) rows, 0 for {0,2}-encoded rows
            wvec16 = cpool.tile([1, 16], dt.float32)
            nc.vector.memset(wvec16[:], 0.0)
            if N_ACT:
                nc.vector.memset(wvec16[:, a0_act:16], 1.0)
            # per-partition bias constants for the ACT Sign h-planes
            bias_act = cpool.tile([P, N_ACT if N_ACT else 1], dt.float32)
            for i in range(N_ACT):
                a = a0_act + i
                nc.vector.memset(bias_act[:, i : i + 1], float(16 * (a + 1)) - 0.5)

            for r in range(rows):
                psums = [
                    ppool.tile([P, P], dt.float32, tag=f"ps{k}", name=f"ps{k}")
                    for k in range(NACC)
                ]
                for t in range(TILES):
                    x16 = x16pool.tile([P, T], dt.int16, tag="x16")
                    if T <= 1024:
                        qs = T // 4
                        x32 = xpool.tile([P, T], dt.int32, tag="x32")
                        for q in range(4):
                            nc.sync.dma_start(
                                out=x32[:, q * qs : (q + 1) * qs],
                                in_=xv[r, t, :, q * qs : (q + 1) * qs],
                            )
                        nc.scalar.copy(out=x16[:], in_=x32[:])
                    else:
                        # halve the staging buffer: two DMA+cast rounds
                        # through one [P, T/2] int32 tile
                        hs = T // 2
                        qs = hs // 2
                        x32 = xpool.tile([P, hs], dt.int32, tag="x32")
                        for h in range(2):
                            for q in range(2):
                                nc.sync.dma_start(
                                    out=x32[:, q * qs : (q + 1) * qs],
                                    in_=xv[
                                        r, t, :,
                                        h * hs + q * qs : h * hs + (q + 1) * qs,
                                    ],
                                )
                            nc.scalar.copy(
                                out=x16[:, h * hs : (h + 1) * hs], in_=x32[:]
                            )

                    # element (p, i) -> group g = i % G, chunk c = i // G
                    xg = x16[:].rearrange("p (c g) -> p c g", g=G)

                    mdt = dt.bfloat16 if MDT == "bf16" else dt.float8e4
                    hm = mpool.tile([P, C, 16, G], mdt, tag="hm")
                    lm = mpool.tile([P, C, 16, G], mdt, tag="lm")

                    # --- l-planes: [x & 15 <= b], 0/1 (hw forbids mixing
                    # bitwise op0 with arith op1, so extract xl first)
                    xl = xlpool.tile([P, T], dt.int16, tag="xl")
                    nc.vector.tensor_scalar(
                        out=xl[:], in0=x16[:], scalar1=15, scalar2=None,
                        op0=alu.bitwise_and,
                    )
                    xlg = xl[:].rearrange("p (c g) -> p c g", g=G)
                    for b in range(16):
                        nc.vector.tensor_scalar(
                            out=lm[:, :, b, :], in0=xlg,
                            scalar1=b, scalar2=None,
                            op0=alu.is_le,
                        )

                    # --- h-planes: [x < 16(a+1)], {0,2}-encoded on DVE/GPS
                    # (uniform epilogue fix: J2 = (J' + w*Sigma_l)/2)
                    for a in range(16):
                        if a < a0_gps:
                            nc.vector.tensor_scalar(
                                out=hm[:, :, a, :], in0=xg,
                                scalar1=16 * (a + 1), scalar2=2,
                                op0=alu.is_lt, op1=alu.mult,
                            )
                        elif a < a0_act:
                            nc.gpsimd.tensor_scalar(
                                out=hm[:, :, a, :], in0=xg,
                                scalar1=16 * (a + 1), scalar2=2,
                                op0=alu.is_lt, op1=alu.mult,
                            )
                        else:
                            # +-1 == Sign(16(a+1) - 0.5 - x); fixed in epilogue
                            nc.scalar.activation(
                                hm[:, :, a, :], xg, act.Sign,
                                bias=bias_act[:, a - a0_act : a - a0_act + 1],
                                scale=-1.0,
                            )

                    for c in range(C):
                        k = c % NACC
                        nc.tensor.matmul(
                            out=psums[k][:],
                            lhsT=hm[:, c, :, :],
                            rhs=lm[:, c, :, :],
                            start=(t == 0 and c < NACC),
                            stop=(t == TILES - 1 and c >= C - NACC),
                        )

                # --- epilogue for row r ---
                # only one PSUM operand allowed per DVE instruction; use the
                # ACT engine for the PSUM drain to keep DVE free
                hsum = epool.tile([P, P], dt.float32, tag="hsum")
                nc.scalar.copy(out=hsum[:], in_=psums[0][:])
                for k in range(1, NACC):
                    nc.vector.tensor_tensor(
                        out=hsum[:], in0=hsum[:], in1=psums[k][:],
                        op=alu.add,
                    )
                hv = hsum[:].rearrange("(a gi) (l gj) -> a gi l gj", gi=G, gj=G)
                tmp = epool.tile([16, 16, G], dt.float32, tag="tmp")
                for g in range(G):
                    nc.sync.dma_start(out=tmp[:, :, g], in_=hv[:, g, :, g])
                jmat = epool.tile([16, 16], dt.float32, tag="jmat")
                nc.vector.tensor_reduce(
                    out=jmat[:], in_=tmp[:], axis=mybir.AxisListType.X,
                    op=alu.add,
                )

                # uniform fix: J2 = (J' + w_a * Sigma_l)/2 with w_a baked
                # into the broadcast matmul stationary. Sigma_l = J'[15,:]
                # (row 15 is the const +1 ACT plane).
                sig = epool.tile([1, 16], dt.float32, tag="sig")
                nc.sync.dma_start(out=sig[:], in_=jmat[15:16, :])
                bc = bcpool.tile([16, 16], dt.float32, tag="bc")
                nc.tensor.matmul(
                    out=bc[:],
                    lhsT=wvec16[:],
                    rhs=sig[:],
                    start=True, stop=True,
                )
                nc.vector.tensor_tensor(
                    out=jmat[:], in0=jmat[:], in1=bc[:], op=alu.add,
                )
                nc.vector.tensor_scalar(
                    out=jmat[:], in0=jmat[:],
                    scalar1=0.5, scalar2=None, op0=alu.mult,
                )

                # difference along a (partition dim): K[a] = J2[a] - J2[a-1]
                jshift = epool.tile([16, 16], dt.float32, tag="jshift")
                nc.vector.memset(jshift[0:1, :], 0.0)
                nc.sync.dma_start(out=jshift[1:16, :], in_=jmat[0:15, :])
                kmat = epool.tile([16, 16], dt.float32, tag="kmat")
                nc.vector.tensor_tensor(
                    out=kmat[:], in0=jmat[:], in1=jshift[:],
                    op=alu.subtract,
                )
                # difference along b (free dim): hist16[:, b] = K[b] - K[b-1]
                hist16 = epool.tile([16, 16], dt.float32, tag="h16")
                nc.vector.tensor_copy(out=hist16[:, 0:1], in_=kmat[:, 0:1])
                nc.vector.tensor_tensor(
                    out=hist16[:, 1:16], in0=kmat[:, 1:16], in1=kmat[:, 0:15],
                    op=alu.subtract,
                )

                histcol = epool.tile([P, 2], dt.float32, tag="hcol")
                nc.sync.dma_start(out=histcol[:, 0:1], in_=hist16[0:8, :])
                nc.sync.dma_start(out=histcol[:, 1:2], in_=hist16[8:16, :])

                for half in range(2):
                    bt = epool.tile([P, LEVELS], dt.float32, tag="bt")
                    nc.scalar.mul(bt[:], ones_t[:], histcol[:, half : half + 1])
                    nc.sync.dma_start(
                        out=ov[r, half * P : (half + 1) * P, :], in_=bt[:]
                    )

    nc.compile()
    return nc


def _get_program(rows=None):
    key = ("nc", rows)
    if key not in _cache:
        _cache[key] = _build_program(rows)
    return _cache[key]


def kernel(x: np.ndarray) -> np.ndarray:
    from concourse.bass_utils import run_bass_kernel_spmd

    x = np.ascontiguousarray(np.asarray(x), dtype=np.int32)
    assert x.shape == (B, N), x.shape

    nc = _get_program()
    in_maps = [
        {"x": x[c * ROWS_PER_CORE : (c + 1) * ROWS_PER_CORE]} for c in range(NCORES)
    ]
    res = run_bass_kernel_spmd(nc, in_maps, core_ids=list(range(NCORES)))
    out = np.concatenate([res.results[c]["out"] for c in range(NCORES)], axis=0)
    return out.astype(np.float32)


# revision 24
# speedup vs baseline: 1.6609x; 1.1254x over previous
"""Per-sample 256-bin histogram -> broadcast [B,256,256], Trainium2 Bass kernel.

Input : x int32 [64, 786432], values in [0, 256)
Output: f32 [64, 256, 256] where out[b, i, j] = count(x[b, :] == i)

Sharding: pure data parallel, 8 rows per core across 8 NeuronCores.

Per-core algorithm (cumulative-threshold decomposition, grouped outer
products, v2):
  J2[a, b]   = sum_n [x_n < 16(a+1)] * [x_n & 15 <= b]   (cumulative in BOTH)
  hist[16a+b] = dda ddb J2  (2-D finite difference)
  Every mask plane is ONE instruction on ONE of three engines:
  - DVE   : tensor_scalar is_lt (h-planes) / chained and+is_le (l-planes),
            int16 in -> bf16 out, 4x mode (~327 ns/plane @ T=1024).
  - ACT   : Sign activation (+-1 encoding, fixed up in the epilogue),
            one op per plane (~1147 ns/plane). ACT also casts the
            contiguously-DMA'd int32 input to int16 (the int16-strided
            DMA variant shatters into 2-byte packets - catastrophic).
  - GPSIMD: same tensor_scalar chains, n_gps planes (port-shared w/ DVE).
  PE accumulates [128,128] PSUM outer products with 8 element groups per
  matmul (1024 elements / matmul instruction).
  Epilogue per row: diag blocks -> J' [16,16]; ACT (+-1) rows fixed via
  J2 = (J' + Sigma_l)/2 where Sigma_l = J'[15,:]; 2-D difference;
  partition-reshape to [128,2]; broadcast multiply; write out.
  Counts are integer-exact in f32 (cumulative counts < 2^24).
"""

import os
import sys

import numpy as np

sys.path.insert(0, "/opt/trn_rl_repo")

B = 64
N = 786432
NCORES = 8
ROWS_PER_CORE = B // NCORES
LEVELS = 256
P = 128

T = int(os.environ.get("K_T", "1024"))  # columns per tile
G = 8  # element groups per matmul
C = T // G  # matmul chunks per tile
TILES = N // (P * T)
assert TILES * P * T == N and C * G == T

NACC = int(os.environ.get("K_NACC", "1"))
N_ACT = int(os.environ.get("K_ACT", "6"))  # h-planes on ACT (Sign, +-1)
N_GPS = int(os.environ.get("K_GPS", "0"))  # h-planes on GPSIMD
MDT = os.environ.get("K_MDT", "bf16")  # mask dtype: bf16 | fp8
assert N_ACT + N_GPS <= 16

_cache = {}


def _build_program(rows=None):
    import concourse.bacc as bacc
    from concourse import mybir
    from concourse import tile

    alu = mybir.AluOpType
    dt = mybir.dt
    act = mybir.ActivationFunctionType

    rows = ROWS_PER_CORE if rows is None else rows

    nc = bacc.Bacc(
        "TRN2",
        target_bir_lowering=False,
        debug=False,
        num_devices=NCORES,
    )
    x_dram = nc.dram_tensor("x", [rows, N], dt.int32, kind="ExternalInput")
    out_dram = nc.dram_tensor(
        "out", [rows, LEVELS, LEVELS], dt.float32, kind="ExternalOutput"
    )

    xv = x_dram.ap().rearrange("r (t p f) -> r t p f", p=P, f=T)
    ov = out_dram.ap()

    # h-plane engine assignment: a = 0..15
    #   DVE  : a in [0, 16-N_ACT-N_GPS)
    #   GPS  : a in [16-N_ACT-N_GPS, 16-N_ACT)
    #   ACT  : a in [16-N_ACT, 16)   (+-1 encoding; includes a=15 == const +1)
    a0_gps = 16 - N_ACT - N_GPS
    a0_act = 16 - N_ACT

    with tile.TileContext(nc) as tc:
        with (
            tc.tile_pool(name="xin", bufs=2 if T <= 1024 else 1) as xpool,
            tc.tile_pool(name="x16", bufs=2) as x16pool,
            tc.tile_pool(name="xl", bufs=2 if T <= 1024 else 1) as xlpool,
            tc.tile_pool(name="mask", bufs=2) as mpool,
            tc.tile_pool(name="acc", bufs=2, space="PSUM") as ppool,
            tc.tile_pool(name="bc", bufs=2, space="PSUM") as bcpool,
            tc.tile_pool(name="epi", bufs=2 if T <= 1024 else 1) as epool,
            tc.tile_pool(name="const", bufs=1) as cpool,
        ):
            ones_t = cpool.tile([P, LEVELS], dt.bfloat16)
            nc.vector.memset(ones_t[:], 1.0)
            # stationary weight row for the epilogue broadcast matmul:
            # w_a = 1 for +-1-encoded (ACT) rows, 0 for {0,2}-encoded rows
            wvec16 = cpool.tile([1, 16], dt.float32)
            nc.vector.memset(wvec16[:], 0.0)
            if N_ACT:
                nc.vector.memset(wvec16[:, a0_act:16], 1.0)
            # per-partition bias constants for the ACT Sign h-planes
            bias_act = cpool.tile([P, N_ACT if N_ACT else 1], dt.float32)
            for i in range(N_ACT):
                a = a0_act + i
                nc.vector.memset(bias_act[:, i : i + 1], float(16 * (a + 1)) - 0.5)

            def load_tile(r, t):
                """DMA + cast one [P, T] input tile; returns the x16 tile."""
                x16 = x16pool.tile([P, T], dt.int16, tag="x16")
                if T <= 1024:
                    qs = T // 4
                    x32 = xpool.tile([P, T], dt.int32, tag="x32")
                    for q in range(4):
                        nc.sync.dma_start(
                            out=x32[:, q * qs : (q + 1) * qs],
                            in_=xv[r, t, :, q * qs : (q + 1) * qs],
                        )
                    nc.scalar.copy(out=x16[:], in_=x32[:])
                else:
                    # halve the staging buffer: two DMA+cast rounds
                    # through one [P, T/2] int32 tile
                    hs = T // 2
                    qs = hs // 2
                    x32 = xpool.tile([P, hs], dt.int32, tag="x32")
                    for h in range(2):
                        for q in range(2):
                            nc.sync.dma_start(
                                out=x32[:, q * qs : (q + 1) * qs],
                                in_=xv[
                                    r, t, :,
                                    h * hs + q * qs : h * hs + (q + 1) * qs,
                                ],
                            )
                        nc.scalar.copy(
                            out=x16[:, h * hs : (h + 1) * hs], in_=x32[:]
                        )
                return x16

            # software pipeline: load (DMA + ACT cast) runs one tile ahead
            # of mask generation so the cast never gates the DVE planes
            steps = [(r, t) for r in range(rows) for t in range(TILES)]
            pend = load_tile(*steps[0])
            for i, (r, t) in enumerate(steps):
                if t == 0:
                    psums = [
                        ppool.tile([P, P], dt.float32, tag=f"ps{k}", name=f"ps{k}")
                        for k in range(NACC)
                    ]
                x16 = pend
                if i + 1 < len(steps):
                    pend = load_tile(*steps[i + 1])
                if True:
                    # element (p, i) -> group g = i % G, chunk c = i // G
                    xg = x16[:].rearrange("p (c g) -> p c g", g=G)

                    mdt = dt.bfloat16 if MDT == "bf16" else dt.float8e4
                    hm = mpool.tile([P, C, 16, G], mdt, tag="hm")
                    lm = mpool.tile([P, C, 16, G], mdt, tag="lm")

                    # --- l-planes: [x & 15 <= b], 0/1 (hw forbids mixing
                    # bitwise op0 with arith op1, so extract xl first)
                    xl = xlpool.tile([P, T], dt.int16, tag="xl")
                    nc.vector.tensor_scalar(
                        out=xl[:], in0=x16[:], scalar1=15, scalar2=None,
                        op0=alu.bitwise_and,
                    )
                    xlg = xl[:].rearrange("p (c g) -> p c g", g=G)
                    for b in range(16):
                        nc.vector.tensor_scalar(
                            out=lm[:, :, b, :], in0=xlg,
                            scalar1=b, scalar2=None,
                            op0=alu.is_le,
                        )

                    # --- h-planes: [x < 16(a+1)], {0,2}-encoded on DVE/GPS
                    # (uniform epilogue fix: J2 = (J' + w*Sigma_l)/2)
                    for a in range(16):
                        if a < a0_gps:
                            nc.vector.tensor_scalar(
                                out=hm[:, :, a, :], in0=xg,
                                scalar1=16 * (a + 1), scalar2=2,
                                op0=alu.is_lt, op1=alu.mult,
                            )
                        elif a < a0_act:
                            nc.gpsimd.tensor_scalar(
                                out=hm[:, :, a, :], in0=xg,
                                scalar1=16 * (a + 1), scalar2=2,
                                op0=alu.is_lt, op1=alu.mult,
                            )
                        else:
                            # +-1 == Sign(16(a+1) - 0.5 - x); fixed in epilogue
                            nc.scalar.activation(
                                hm[:, :, a, :], xg, act.Sign,
                                bias=bias_act[:, a - a0_act : a - a0_act + 1],
                                scale=-1.0,
                            )

                    for c in range(C):
                        k = c % NACC
                        nc.tensor.matmul(
                            out=psums[k][:],
                            lhsT=hm[:, c, :, :],
                            rhs=lm[:, c, :, :],
                            start=(t == 0 and c < NACC),
                            stop=(t == TILES - 1 and c >= C - NACC),
                        )

                if t != TILES - 1:
                    continue
                # --- epilogue for row r ---
                # only one PSUM operand allowed per DVE instruction; use the
                # ACT engine for the PSUM drain to keep DVE free
                hsum = epool.tile([P, P], dt.float32, tag="hsum")
                nc.scalar.copy(out=hsum[:], in_=psums[0][:])
                for k in range(1, NACC):
                    nc.vector.tensor_tensor(
                        out=hsum[:], in0=hsum[:], in1=psums[k][:],
                        op=alu.add,
                    )
                hv = hsum[:].rearrange("(a gi) (l gj) -> a gi l gj", gi=G, gj=G)
                tmp = epool.tile([16, 16, G], dt.float32, tag="tmp")
                for g in range(G):
                    nc.sync.dma_start(out=tmp[:, :, g], in_=hv[:, g, :, g])
                jmat = epool.tile([16, 16], dt.float32, tag="jmat")
                nc.vector.tensor_reduce(
                    out=jmat[:], in_=tmp[:], axis=mybir.AxisListType.X,
                    op=alu.add,
                )

                # uniform fix: J2 = (J' + w_a * Sigma_l)/2 with w_a baked
                # into the broadcast matmul stationary. Sigma_l = J'[15,:]
                # (row 15 is the const +1 ACT plane).
                sig = epool.tile([1, 16], dt.float32, tag="sig")
                nc.sync.dma_start(out=sig[:], in_=jmat[15:16, :])
                bc = bcpool.tile([16, 16], dt.float32, tag="bc")
                nc.tensor.matmul(
                    out=bc[:],
                    lhsT=wvec16[:],
                    rhs=sig[:],
                    start=True, stop=True,
                )
                nc.vector.tensor_tensor(
                    out=jmat[:], in0=jmat[:], in1=bc[:], op=alu.add,
                )
                nc.vector.tensor_scalar(
                    out=jmat[:], in0=jmat[:],
                    scalar1=0.5, scalar2=None, op0=alu.mult,
                )

                # difference along a (partition dim): K[a] = J2[a] - J2[a-1]
                jshift = epool.tile([16, 16], dt.float32, tag="jshift")
                nc.vector.memset(jshift[0:1, :], 0.0)
                nc.sync.dma_start(out=jshift[1:16, :], in_=jmat[0:15, :])
                kmat = epool.tile([16, 16], dt.float32, tag="kmat")
                nc.vector.tensor_tensor(
                    out=kmat[:], in0=jmat[:], in1=jshift[:],
                    op=alu.subtract,
                )
                # difference along b (free dim): hist16[:, b] = K[b] - K[b-1]
                hist16 = epool.tile([16, 16], dt.float32, tag="h16")
                nc.vector.tensor_copy(out=hist16[:, 0:1], in_=kmat[:, 0:1])
                nc.vector.tensor_tensor(
                    out=hist16[:, 1:16], in0=kmat[:, 1:16], in1=kmat[:, 0:15],
                    op=alu.subtract,
                )

                histcol = epool.tile([P, 2], dt.float32, tag="hcol")
                nc.sync.dma_start(out=histcol[:, 0:1], in_=hist16[0:8, :])
                nc.sync.dma_start(out=histcol[:, 1:2], in_=hist16[8:16, :])

                for half in range(2):
                    bt = epool.tile([P, LEVELS], dt.float32, tag="bt")
                    nc.scalar.mul(bt[:], ones_t[:], histcol[:, half : half + 1])
                    nc.sync.dma_start(
                        out=ov[r, half * P : (half + 1) * P, :], in_=bt[:]
                    )

    nc.compile()
    return nc


def _get_program(rows=None):
    key = ("nc", rows)
    if key not in _cache:
        _cache[key] = _build_program(rows)
    return _cache[key]


def kernel(x: np.ndarray) -> np.ndarray:
    from concourse.bass_utils import run_bass_kernel_spmd

    x = np.ascontiguousarray(np.asarray(x), dtype=np.int32)
    assert x.shape == (B, N), x.shape

    nc = _get_program()
    in_maps = [
        {"x": x[c * ROWS_PER_CORE : (c + 1) * ROWS_PER_CORE]} for c in range(NCORES)
    ]
    res = run_bass_kernel_spmd(nc, in_maps, core_ids=list(range(NCORES)))
    out = np.concatenate([res.results[c]["out"] for c in range(NCORES)], axis=0)
    return out.astype(np.float32)


# revision 27
# speedup vs baseline: 1.6650x; 1.0024x over previous
"""Per-sample 256-bin histogram -> broadcast [B,256,256], Trainium2 Bass kernel.

Input : x int32 [64, 786432], values in [0, 256)
Output: f32 [64, 256, 256] where out[b, i, j] = count(x[b, :] == i)

Sharding: pure data parallel, 8 rows per core across 8 NeuronCores.

Per-core algorithm (cumulative-threshold decomposition, grouped outer
products, v2):
  J2[a, b]   = sum_n [x_n < 16(a+1)] * [x_n & 15 <= b]   (cumulative in BOTH)
  hist[16a+b] = dda ddb J2  (2-D finite difference)
  Every mask plane is ONE instruction, split across two engines:
  - DVE : 16 l-planes (is_le on the extracted low nibble) + 10 h-planes
          (chained is_lt*2 -> {0,2} encoding) + 1 nibble extract,
          int16 in -> bf16 out, 4x mode (~330 ns/plane @ T=1024).
  - ACT : 6 h-planes as single-op Sign activations (+-1 encoding) plus
          the int32->int16 input cast. Input is DMA'd as CONTIGUOUS
          int32 (a strided int16 DMA shatters into 2-byte packets and
          is catastrophically slow).
  - GPSIMD is left idle on purpose: its tensor_scalar is ~17x slower
    and it contends with DVE for the shared SBUF port.
  The DMA+cast runs one tile ahead of mask generation (software
  pipeline). PE accumulates [128,128] PSUM outer products with 8
  element groups per matmul (1024 elements / matmul instruction).
  Epilogue per row: diag blocks -> J' [16,16]; the mixed encodings are
  fixed uniformly via J2 = (J' + w_a*Sigma_l)/2 with w_a in {0,1} baked
  into a tiny broadcast matmul (Sigma_l = J'[15,:]); 2-D difference;
  partition-reshape to [128,2]; broadcast multiply; write out.
  Counts are integer-exact in f32 (cumulative counts < 2^24).

Measured on the 8-core harness: ~501 us HW exec (baseline 547 us);
engines: DVE ~443 us, ACT ~369 us, PE ~373 us busy -> DVE-bound at its
write-port floor for 27 plane-ops/tile. fp8 masks (drops DVE to 2x),
GPSIMD offload, and T=1536 (SBUF-starved, stalls) were all tried and
are slower; env knobs K_T/K_ACT/K_GPS/K_NACC/K_MDT remain for tuning.
"""

import os
import sys

import numpy as np

sys.path.insert(0, "/opt/trn_rl_repo")

B = 64
N = 786432
NCORES = 8
ROWS_PER_CORE = B // NCORES
LEVELS = 256
P = 128

T = int(os.environ.get("K_T", "1024"))  # columns per tile
G = 8  # element groups per matmul
C = T // G  # matmul chunks per tile
TILES = N // (P * T)
assert TILES * P * T == N and C * G == T

NACC = int(os.environ.get("K_NACC", "1"))
N_ACT = int(os.environ.get("K_ACT", "6"))  # h-planes on ACT (Sign, +-1)
N_GPS = int(os.environ.get("K_GPS", "0"))  # h-planes on GPSIMD
MDT = os.environ.get("K_MDT", "bf16")  # mask dtype: bf16 | fp8
assert N_ACT + N_GPS <= 16

_cache = {}


def _build_program(rows=None):
    import concourse.bacc as bacc
    from concourse import mybir
    from concourse import tile

    alu = mybir.AluOpType
    dt = mybir.dt
    act = mybir.ActivationFunctionType

    rows = ROWS_PER_CORE if rows is None else rows

    nc = bacc.Bacc(
        "TRN2",
        target_bir_lowering=False,
        debug=False,
        num_devices=NCORES,
    )
    x_dram = nc.dram_tensor("x", [rows, N], dt.int32, kind="ExternalInput")
    out_dram = nc.dram_tensor(
        "out", [rows, LEVELS, LEVELS], dt.float32, kind="ExternalOutput"
    )

    xv = x_dram.ap().rearrange("r (t p f) -> r t p f", p=P, f=T)
    ov = out_dram.ap()

    # h-plane engine assignment: a = 0..15
    #   DVE  : a in [0, 16-N_ACT-N_GPS)
    #   GPS  : a in [16-N_ACT-N_GPS, 16-N_ACT)
    #   ACT  : a in [16-N_ACT, 16)   (+-1 encoding; includes a=15 == const +1)
    a0_gps = 16 - N_ACT - N_GPS
    a0_act = 16 - N_ACT

    with tile.TileContext(nc) as tc:
        with (
            tc.tile_pool(name="xin", bufs=2 if T <= 1024 else 1) as xpool,
            tc.tile_pool(name="x16", bufs=2) as x16pool,
            tc.tile_pool(name="xl", bufs=2 if T <= 1024 else 1) as xlpool,
            tc.tile_pool(name="mask", bufs=2) as mpool,
            tc.tile_pool(name="acc", bufs=2, space="PSUM") as ppool,
            tc.tile_pool(name="bc", bufs=2, space="PSUM") as bcpool,
            tc.tile_pool(name="epi", bufs=2 if T <= 1024 else 1) as epool,
            tc.tile_pool(name="const", bufs=1) as cpool,
        ):
            ones_t = cpool.tile([P, LEVELS], dt.bfloat16)
            nc.vector.memset(ones_t[:], 1.0)
            # stationary weight row for the epilogue broadcast matmul:
            # w_a = 1 for +-1-encoded (ACT) rows, 0 for {0,2}-encoded rows
            wvec16 = cpool.tile([1, 16], dt.float32)
            nc.vector.memset(wvec16[:], 0.0)
            if N_ACT:
                nc.vector.memset(wvec16[:, a0_act:16], 1.0)
            # per-partition bias constants for the ACT Sign h-planes
            bias_act = cpool.tile([P, N_ACT if N_ACT else 1], dt.float32)
            for i in range(N_ACT):
                a = a0_act + i
                nc.vector.memset(bias_act[:, i : i + 1], float(16 * (a + 1)) - 0.5)

            def load_tile(r, t):
                """DMA + cast one [P, T] input tile; returns the x16 tile."""
                x16 = x16pool.tile([P, T], dt.int16, tag="x16")
                if T <= 1024:
                    qs = T // 4
                    x32 = xpool.tile([P, T], dt.int32, tag="x32")
                    for q in range(4):
                        nc.sync.dma_start(
                            out=x32[:, q * qs : (q + 1) * qs],
                            in_=xv[r, t, :, q * qs : (q + 1) * qs],
                        )
                    nc.scalar.copy(out=x16[:], in_=x32[:])
                else:
                    # halve the staging buffer: two DMA+cast rounds
                    # through one [P, T/2] int32 tile
                    hs = T // 2
                    qs = hs // 2
                    x32 = xpool.tile([P, hs], dt.int32, tag="x32")
                    for h in range(2):
                        for q in range(2):
                            nc.sync.dma_start(
                                out=x32[:, q * qs : (q + 1) * qs],
                                in_=xv[
                                    r, t, :,
                                    h * hs + q * qs : h * hs + (q + 1) * qs,
                                ],
                            )
                        nc.scalar.copy(
                            out=x16[:, h * hs : (h + 1) * hs], in_=x32[:]
                        )
                return x16

            # software pipeline: load (DMA + ACT cast) runs one tile ahead
            # of mask generation so the cast never gates the DVE planes
            steps = [(r, t) for r in range(rows) for t in range(TILES)]
            pend = load_tile(*steps[0])
            for i, (r, t) in enumerate(steps):
                if t == 0:
                    psums = [
                        ppool.tile([P, P], dt.float32, tag=f"ps{k}", name=f"ps{k}")
                        for k in range(NACC)
                    ]
                x16 = pend
                if i + 1 < len(steps):
                    pend = load_tile(*steps[i + 1])
                if True:
                    # element (p, i) -> group g = i % G, chunk c = i // G
                    xg = x16[:].rearrange("p (c g) -> p c g", g=G)

                    mdt = dt.bfloat16 if MDT == "bf16" else dt.float8e4
                    hm = mpool.tile([P, C, 16, G], mdt, tag="hm")
                    lm = mpool.tile([P, C, 16, G], mdt, tag="lm")

                    # --- l-planes: [x & 15 <= b], 0/1 (hw forbids mixing
                    # bitwise op0 with arith op1, so extract xl first)
                    xl = xlpool.tile([P, T], dt.int16, tag="xl")
                    nc.vector.tensor_scalar(
                        out=xl[:], in0=x16[:], scalar1=15, scalar2=None,
                        op0=alu.bitwise_and,
                    )
                    xlg = xl[:].rearrange("p (c g) -> p c g", g=G)
                    for b in range(16):
                        nc.vector.tensor_scalar(
                            out=lm[:, :, b, :], in0=xlg,
                            scalar1=b, scalar2=None,
                            op0=alu.is_le,
                        )

                    # --- h-planes: [x < 16(a+1)], {0,2}-encoded on DVE/GPS
                    # (uniform epilogue fix: J2 = (J' + w*Sigma_l)/2)
                    for a in range(16):
                        if a < a0_gps:
                            nc.vector.tensor_scalar(
                                out=hm[:, :, a, :], in0=xg,
                                scalar1=16 * (a + 1), scalar2=2,
                                op0=alu.is_lt, op1=alu.mult,
                            )
                        elif a < a0_act:
                            nc.gpsimd.tensor_scalar(
                                out=hm[:, :, a, :], in0=xg,
                                scalar1=16 * (a + 1), scalar2=2,
                                op0=alu.is_lt, op1=alu.mult,
                            )
                        else:
                            # +-1 == Sign(16(a+1) - 0.5 - x); fixed in epilogue
                            nc.scalar.activation(
                                hm[:, :, a, :], xg, act.Sign,
                                bias=bias_act[:, a - a0_act : a - a0_act + 1],
                                scale=-1.0,
                            )

                    for c in range(C):
                        k = c % NACC
                        nc.tensor.matmul(
                            out=psums[k][:],
                            lhsT=hm[:, c, :, :],
                            rhs=lm[:, c, :, :],
                            start=(t == 0 and c < NACC),
                            stop=(t == TILES - 1 and c >= C - NACC),
                        )

                if t != TILES - 1:
                    continue
                # --- epilogue for row r ---
                # only one PSUM operand allowed per DVE instruction; use the
                # ACT engine for the PSUM drain to keep DVE free
                hsum = epool.tile([P, P], dt.float32, tag="hsum")
                nc.scalar.copy(out=hsum[:], in_=psums[0][:])
                for k in range(1, NACC):
                    nc.vector.tensor_tensor(
                        out=hsum[:], in0=hsum[:], in1=psums[k][:],
                        op=alu.add,
                    )
                hv = hsum[:].rearrange("(a gi) (l gj) -> a gi l gj", gi=G, gj=G)
                tmp = epool.tile([16, 16, G], dt.float32, tag="tmp")
                for g in range(G):
                    nc.sync.dma_start(out=tmp[:, :, g], in_=hv[:, g, :, g])
                jmat = epool.tile([16, 16], dt.float32, tag="jmat")
                nc.vector.tensor_reduce(
                    out=jmat[:], in_=tmp[:], axis=mybir.AxisListType.X,
                    op=alu.add,
                )

                # uniform fix: J2 = (J' + w_a * Sigma_l)/2 with w_a baked
                # into the broadcast matmul stationary. Sigma_l = J'[15,:]
                # (row 15 is the const +1 ACT plane).
                sig = epool.tile([1, 16], dt.float32, tag="sig")
                nc.sync.dma_start(out=sig[:], in_=jmat[15:16, :])
                bc = bcpool.tile([16, 16], dt.float32, tag="bc")
                nc.tensor.matmul(
                    out=bc[:],
                    lhsT=wvec16[:],
                    rhs=sig[:],
                    start=True, stop=True,
                )
                nc.vector.tensor_tensor(
                    out=jmat[:], in0=jmat[:], in1=bc[:], op=alu.add,
                )
                nc.vector.tensor_scalar(
                    out=jmat[:], in0=jmat[:],
                    scalar1=0.5, scalar2=None, op0=alu.mult,
                )

                # difference along a (partition dim): K[a] = J2[a] - J2[a-1]
                jshift = epool.tile([16, 16], dt.float32, tag="jshift")
                nc.vector.memset(jshift[0:1, :], 0.0)
                nc.sync.dma_start(out=jshift[1:16, :], in_=jmat[0:15, :])
                kmat = epool.tile([16, 16], dt.float32, tag="kmat")
                nc.vector.tensor_tensor(
                    out=kmat[:], in0=jmat[:], in1=jshift[:],
                    op=alu.subtract,
                )
                # difference along b (free dim): hist16[:, b] = K[b] - K[b-1]
                hist16 = epool.tile([16, 16], dt.float32, tag="h16")
                nc.vector.tensor_copy(out=hist16[:, 0:1], in_=kmat[:, 0:1])
                nc.vector.tensor_tensor(
                    out=hist16[:, 1:16], in0=kmat[:, 1:16], in1=kmat[:, 0:15],
                    op=alu.subtract,
                )

                histcol = epool.tile([P, 2], dt.float32, tag="hcol")
                nc.sync.dma_start(out=histcol[:, 0:1], in_=hist16[0:8, :])
                nc.sync.dma_start(out=histcol[:, 1:2], in_=hist16[8:16, :])

                for half in range(2):
                    bt = epool.tile([P, LEVELS], dt.float32, tag="bt")
                    nc.scalar.mul(bt[:], ones_t[:], histcol[:, half : half + 1])
                    nc.sync.dma_start(
                        out=ov[r, half * P : (half + 1) * P, :], in_=bt[:]
                    )

    nc.compile()
    return nc


def _get_program(rows=None):
    key = ("nc", rows)
    if key not in _cache:
        _cache[key] = _build_program(rows)
    return _cache[key]


def kernel(x: np.ndarray) -> np.ndarray:
    from concourse.bass_utils import run_bass_kernel_spmd

    x = np.ascontiguousarray(np.asarray(x), dtype=np.int32)
    assert x.shape == (B, N), x.shape

    nc = _get_program()
    in_maps = [
        {"x": x[c * ROWS_PER_CORE : (c + 1) * ROWS_PER_CORE]} for c in range(NCORES)
    ]
    res = run_bass_kernel_spmd(nc, in_maps, core_ids=list(range(NCORES)))
    out = np.concatenate([res.results[c]["out"] for c in range(NCORES)], axis=0)
    return out.astype(np.float32)
